# revision 18
# baseline (speedup 1.0000x reference)
"""Bass/Trainium2 kernel for batched int8 matmul with fp32 dequant epilogue.

Computes out[b, m, n] = alpha * sum_k a[b, m, k] * b[b, n, k] for
a, b int8 [256, 512, 128], out fp32 [256, 512, 512].

Strategy:
  - Shard the batch dim B=256 across 8 NeuronCores (32 batches/core).
  - int8 values convert EXACTLY to bf16 (8-bit significand covers +-256);
    products are ints <= 2^14 and the K=128 accumulation stays <= 2^21,
    exactly representable in the fp32 PSUM accumulator -> the bf16 matmul
    reproduces the int32-accumulated reference bit-exactly.
  - Host pre-transposes both operands to [B, K, M/N] so K lands on the
    SBUF partition dim (the PE contracts over partitions) with fully
    contiguous DMA rows.
  - K=128 means each [128m x 512n] output tile is a single matmul
    (no accumulation loop). alpha is folded into the PSUM->SBUF copy,
    alternating ScalarE/VectorE; fp32 out DMAs back to HBM.
"""

import os
import sys

import numpy as np

B, M, N, K = 256, 512, 512, 128
NCORES = 8
BPC = B // NCORES  # batches per core
MT = M // 128  # m-tiles per batch
OG = 2  # batches per output DMA group (2 batches -> 2 MiB per dma_start)
IG = 4  # batches per input DMA chunk (4 batches -> 1 MiB per dma_start)

_cache = {}
LAST_RESULTS = None  # BassKernelResults of the most recent run (for profiling)


def _build(alpha: float):
    from contextlib import ExitStack

    import concourse.bass as bass
    import concourse.mybir as mybir
    import concourse.tile as tile
    from concourse import bacc

    nc = bacc.Bacc("TRN2", debug=False, enable_asserts=False, num_devices=NCORES)
    # a and b packed along the free dim so one DMA region feeds both matmul
    # operands. Shipped as int8 (half the HBM read traffic); the SWDGE
    # input DMA casts int8 -> bf16 on the fly.
    ab = nc.dram_tensor("ab", [BPC, K, M + N], mybir.dt.int8, kind="ExternalInput")
    out = nc.dram_tensor("out", [BPC, M, N], mybir.dt.float32, kind="ExternalOutput")

    ap_ab = ab.ap()
    # DRAM out viewed with the partition dim innermost of the row index:
    # [BPC, (t p), n] -> [g, p, i, t, n] so one DMA writes OG whole batches
    # from an SBUF tile laid out [p, i, t, n].
    ap_o = out.ap().rearrange("(g i) (t p) n -> g p i t n", p=128, i=OG)

    with ExitStack() as ctx:
        tc = ctx.enter_context(tile.TileContext(nc))
        ab_pool = ctx.enter_context(tc.tile_pool(name="ab", bufs=1))
        ps_pool = ctx.enter_context(tc.tile_pool(name="ps", bufs=8, space="PSUM"))
        o_pool = ctx.enter_context(tc.tile_pool(name="o", bufs=3))

        # Whole input resident in SBUF (64KB/partition), streamed in as
        # chunks so the first matmuls start early. gpsimd (SWDGE) DMAs
        # cast int8 -> bf16 inline and use rings separate from the two
        # HWDGE output queues.
        ab_sb = ab_pool.tile([K, BPC, M + N], mybir.dt.bfloat16, tag="ab")
        for c0 in range(0, BPC, IG):
            nc.gpsimd.dma_start(
                ab_sb[:, c0 : c0 + IG, :],
                ap_ab[c0 : c0 + IG].rearrange("i k f -> k i f"),
            )

        for g in range(BPC // OG):
            o_sb = o_pool.tile([128, OG, MT, N], mybir.dt.float32, tag="o")
            for gi in range(OG):
                i = g * OG + gi
                for mt in range(MT):
                    ps = ps_pool.tile([128, N], mybir.dt.float32, tag="ps")
                    nc.tensor.matmul(
                        ps[:],
                        ab_sb[:, i, mt * 128 : (mt + 1) * 128],
                        ab_sb[:, i, M : M + N],
                        start=True,
                        stop=True,
                    )
                    # Epilogue split across ScalarE and VectorE (each alone
                    # saturates; together they hide under the out-DMA stream).
                    dst = o_sb[:, gi, mt, :]
                    if (i * MT + mt) % 2 == 0:
                        nc.scalar.mul(dst, ps[:], float(alpha))
                    else:
                        nc.vector.tensor_scalar_mul(dst, ps[:], float(alpha))
            # Alternate output DMAs across the two HWDGE queues.
            if g % 2 == 0:
                nc.scalar.dma_start(ap_o[g], o_sb[:])
            else:
                nc.sync.dma_start(ap_o[g], o_sb[:])
    nc.compile()
    return nc


def _get_nc(alpha: float):
    key = np.float32(alpha).tobytes()
    if key not in _cache:
        _cache[key] = _build(alpha)
    return _cache[key]


def kernel(a, b, alpha):
    import ml_dtypes

    from concourse.bass_utils import run_bass_kernel_spmd

    global LAST_RESULTS

    a = np.asarray(a)
    b = np.asarray(b)
    alpha_f = float(np.float32(np.asarray(alpha)))

    # Transpose-pack as int8 so K is the leading (partition) dim on
    # device: [B, M, K] -> [B, K, M]; a and b side by side along the free
    # dim. The device DMA casts int8 -> bf16 (exact for |v| <= 128).
    abT = np.empty((B, K, M + N), dtype=np.int8)
    abT[:, :, :M] = np.swapaxes(a, 1, 2).astype(np.int8, copy=False)
    abT[:, :, M:] = np.swapaxes(b, 1, 2).astype(np.int8, copy=False)

    nc = _get_nc(alpha_f)
    in_maps = [
        {"ab": abT[c * BPC : (c + 1) * BPC]}
        for c in range(NCORES)
    ]
    res = run_bass_kernel_spmd(nc, in_maps, core_ids=list(range(NCORES)))
    LAST_RESULTS = res
    return np.concatenate([r["out"] for r in res.results], axis=0)


# revision 22
# speedup vs baseline: 1.0464x; 1.0464x over previous
"""Bass/Trainium2 kernel for batched int8 matmul with fp32 dequant epilogue.

Computes out[b, m, n] = alpha * sum_k a[b, m, k] * b[b, n, k] for
a, b int8 [256, 512, 128], out fp32 [256, 512, 512].

Strategy:
  - Shard the batch dim B=256 across 8 NeuronCores (32 batches/core).
  - int8 values convert EXACTLY to bf16 (8-bit significand covers +-256);
    products are ints <= 2^14 and the K=128 accumulation stays <= 2^21,
    exactly representable in the fp32 PSUM accumulator -> the bf16 matmul
    reproduces the int32-accumulated reference bit-exactly.
  - Host pre-transposes both operands to [B, K, M/N] so K lands on the
    SBUF partition dim (the PE contracts over partitions) with fully
    contiguous DMA rows.
  - K=128 means each [128m x 512n] output tile is a single matmul
    (no accumulation loop). alpha is folded into the PSUM->SBUF copy,
    alternating ScalarE/VectorE; fp32 out DMAs back to HBM.
"""

import os
import sys

import numpy as np

B, M, N, K = 256, 512, 512, 128
NCORES = 8
BPC = B // NCORES  # batches per core
MT = M // 128  # m-tiles per batch
OG = 2  # batches per output DMA group (2 batches -> 2 MiB per dma_start)
IG = 4  # batches per input DMA chunk (4 batches -> 1 MiB per dma_start)
HEAD = 4  # leading batches shipped as bf16 and loaded via fast HWDGE

_cache = {}
LAST_RESULTS = None  # BassKernelResults of the most recent run (for profiling)


def _build(alpha: float):
    from contextlib import ExitStack

    import concourse.bass as bass
    import concourse.mybir as mybir
    import concourse.tile as tile
    from concourse import bacc

    nc = bacc.Bacc("TRN2", debug=False, enable_asserts=False, num_devices=NCORES)
    # a and b packed along the free dim so one DMA region feeds both matmul
    # operands. The first HEAD batches ship as bf16 and load via HWDGE
    # (sub-us first byte, so matmuls start ~4us in); the rest ships as
    # int8 (half the HBM read traffic), cast to bf16 inline by the SWDGE
    # input DMAs.
    abh = nc.dram_tensor(
        "abh", [HEAD, K, M + N], mybir.dt.bfloat16, kind="ExternalInput"
    )
    abt = nc.dram_tensor(
        "abt", [BPC - HEAD, K, M + N], mybir.dt.int8, kind="ExternalInput"
    )
    out = nc.dram_tensor("out", [BPC, M, N], mybir.dt.float32, kind="ExternalOutput")

    ap_abh = abh.ap()
    ap_abt = abt.ap()
    # DRAM out viewed with the partition dim innermost of the row index:
    # [BPC, (t p), n] -> [g, p, i, t, n] so one DMA writes OG whole batches
    # from an SBUF tile laid out [p, i, t, n].
    ap_o = out.ap().rearrange("(g i) (t p) n -> g p i t n", p=128, i=OG)

    with ExitStack() as ctx:
        tc = ctx.enter_context(tile.TileContext(nc))
        ab_pool = ctx.enter_context(tc.tile_pool(name="ab", bufs=1))
        ps_pool = ctx.enter_context(tc.tile_pool(name="ps", bufs=8, space="PSUM"))
        o_pool = ctx.enter_context(tc.tile_pool(name="o", bufs=3))

        # Whole input resident in SBUF (64KB/partition), streamed in as
        # chunks so the first matmuls start early. The bf16 head goes via
        # HWDGE; the int8 tail via gpsimd (SWDGE) with inline cast, on
        # rings separate from the two HWDGE output queues.
        ab_sb = ab_pool.tile([K, BPC, M + N], mybir.dt.bfloat16, tag="ab")
        nc.sync.dma_start(
            ab_sb[:, 0:HEAD, :], ap_abh.rearrange("i k f -> k i f")
        )
        for c0 in range(0, BPC - HEAD, IG):
            nc.gpsimd.dma_start(
                ab_sb[:, HEAD + c0 : HEAD + c0 + IG, :],
                ap_abt[c0 : c0 + IG].rearrange("i k f -> k i f"),
            )

        for g in range(BPC // OG):
            o_sb = o_pool.tile([128, OG, MT, N], mybir.dt.float32, tag="o")
            for gi in range(OG):
                i = g * OG + gi
                for mt in range(MT):
                    ps = ps_pool.tile([128, N], mybir.dt.float32, tag="ps")
                    nc.tensor.matmul(
                        ps[:],
                        ab_sb[:, i, mt * 128 : (mt + 1) * 128],
                        ab_sb[:, i, M : M + N],
                        start=True,
                        stop=True,
                    )
                    # Epilogue split across ScalarE and VectorE (each alone
                    # saturates; together they hide under the out-DMA stream).
                    dst = o_sb[:, gi, mt, :]
                    if (i * MT + mt) % 2 == 0:
                        nc.scalar.mul(dst, ps[:], float(alpha))
                    else:
                        nc.vector.tensor_scalar_mul(dst, ps[:], float(alpha))
            # Alternate output DMAs across the two HWDGE queues.
            if g % 2 == 0:
                nc.scalar.dma_start(ap_o[g], o_sb[:])
            else:
                nc.sync.dma_start(ap_o[g], o_sb[:])
    nc.compile()
    return nc


def _get_nc(alpha: float):
    key = np.float32(alpha).tobytes()
    if key not in _cache:
        _cache[key] = _build(alpha)
    return _cache[key]


def kernel(a, b, alpha):
    import ml_dtypes

    from concourse.bass_utils import run_bass_kernel_spmd

    global LAST_RESULTS

    a = np.asarray(a)
    b = np.asarray(b)
    alpha_f = float(np.float32(np.asarray(alpha)))

    # Transpose-pack as int8 so K is the leading (partition) dim on
    # device: [B, M, K] -> [B, K, M]; a and b side by side along the free
    # dim. The device DMA casts int8 -> bf16 (exact for |v| <= 128); the
    # per-core HEAD batches ship pre-cast to bf16 for a fast HWDGE start.
    abT = np.empty((B, K, M + N), dtype=np.int8)
    abT[:, :, :M] = np.swapaxes(a, 1, 2).astype(np.int8, copy=False)
    abT[:, :, M:] = np.swapaxes(b, 1, 2).astype(np.int8, copy=False)

    nc = _get_nc(alpha_f)
    in_maps = [
        {
            "abh": abT[c * BPC : c * BPC + HEAD].astype(ml_dtypes.bfloat16),
            "abt": abT[c * BPC + HEAD : (c + 1) * BPC],
        }
        for c in range(NCORES)
    ]
    res = run_bass_kernel_spmd(nc, in_maps, core_ids=list(range(NCORES)))
    LAST_RESULTS = res
    return np.concatenate([r["out"] for r in res.results], axis=0)


# revision 23
# speedup vs baseline: 1.0802x; 1.0323x over previous
"""Bass/Trainium2 kernel for batched int8 matmul with fp32 dequant epilogue.

Computes out[b, m, n] = alpha * sum_k a[b, m, k] * b[b, n, k] for
a, b int8 [256, 512, 128], out fp32 [256, 512, 512].

Strategy:
  - Shard the batch dim B=256 across 8 NeuronCores (32 batches/core).
  - int8 values convert EXACTLY to bf16 (8-bit significand covers +-256);
    products are ints <= 2^14 and the K=128 accumulation stays <= 2^21,
    exactly representable in the fp32 PSUM accumulator -> the bf16 matmul
    reproduces the int32-accumulated reference bit-exactly.
  - Host pre-transposes both operands to [B, K, M/N] so K lands on the
    SBUF partition dim (the PE contracts over partitions) with fully
    contiguous DMA rows.
  - K=128 means each [128m x 512n] output tile is a single matmul
    (no accumulation loop). alpha is folded into the PSUM->SBUF copy,
    alternating ScalarE/VectorE; fp32 out DMAs back to HBM.
"""

import os
import sys

import numpy as np

B, M, N, K = 256, 512, 512, 128
NCORES = 8
BPC = B // NCORES  # batches per core
MT = M // 128  # m-tiles per batch
OG = 2  # batches per output DMA group (2 batches -> 2 MiB per dma_start)
IG = 4  # batches per input DMA chunk (4 batches -> 1 MiB per dma_start)
HEAD = 4  # leading batches shipped as bf16 and loaded via fast HWDGE

_cache = {}
LAST_RESULTS = None  # BassKernelResults of the most recent run (for profiling)


def _build(alpha: float):
    from contextlib import ExitStack

    import concourse.bass as bass
    import concourse.mybir as mybir
    import concourse.tile as tile
    from concourse import bacc

    nc = bacc.Bacc("TRN2", debug=False, enable_asserts=False, num_devices=NCORES)
    # a and b packed along the free dim so one DMA region feeds both matmul
    # operands. The first HEAD batches ship as bf16 and load via HWDGE
    # (sub-us first byte, so matmuls start ~4us in); the rest ships as
    # int8 (half the HBM read traffic), cast to bf16 inline by the SWDGE
    # input DMAs.
    abh = nc.dram_tensor(
        "abh", [HEAD, K, M + N], mybir.dt.bfloat16, kind="ExternalInput"
    )
    abt = nc.dram_tensor(
        "abt", [BPC - HEAD, K, M + N], mybir.dt.int8, kind="ExternalInput"
    )
    out = nc.dram_tensor("out", [BPC, M, N], mybir.dt.float32, kind="ExternalOutput")

    ap_abh = abh.ap()
    ap_abt = abt.ap()
    # DRAM out viewed with the partition dim innermost of the row index:
    # [BPC, (t p), n] -> [g, p, i, t, n] so one DMA writes OG whole batches
    # from an SBUF tile laid out [p, i, t, n].
    ap_o = out.ap().rearrange("(g i) (t p) n -> g p i t n", p=128, i=OG)

    with ExitStack() as ctx:
        tc = ctx.enter_context(tile.TileContext(nc))
        ab_pool = ctx.enter_context(tc.tile_pool(name="ab", bufs=1))
        ps_pool = ctx.enter_context(tc.tile_pool(name="ps", bufs=7, space="PSUM"))
        wm_pool = ctx.enter_context(tc.tile_pool(name="wm", bufs=1, space="PSUM"))
        wms_pool = ctx.enter_context(tc.tile_pool(name="wms", bufs=1))
        o_pool = ctx.enter_context(tc.tile_pool(name="o", bufs=4))

        # ~4us of dummy back-to-back matmuls at t0 (PE is idle while the
        # first input chunk streams in anyway) to lift the PE HAM clock
        # gate from 1.2 to 2.4 GHz; the steady-state matmul stream then
        # keeps it warm. Cold MMs would otherwise pace the whole pipeline.
        wm_sb = wms_pool.tile([K, 128], mybir.dt.bfloat16, tag="wms")
        nc.vector.memset(wm_sb[:], 0)
        wm_ps = wm_pool.tile([128, 512], mybir.dt.float32, tag="wm")
        for _ in range(40):
            nc.tensor.matmul(
                wm_ps[:, 0:128], wm_sb[:], wm_sb[:], start=True, stop=True
            )

        # Whole input resident in SBUF (64KB/partition), streamed in as
        # chunks so the first matmuls start early. The bf16 head goes via
        # HWDGE; the int8 tail via gpsimd (SWDGE) with inline cast, on
        # rings separate from the two HWDGE output queues.
        ab_sb = ab_pool.tile([K, BPC, M + N], mybir.dt.bfloat16, tag="ab")
        nc.sync.dma_start(
            ab_sb[:, 0:HEAD, :], ap_abh.rearrange("i k f -> k i f")
        )
        for c0 in range(0, BPC - HEAD, IG):
            nc.gpsimd.dma_start(
                ab_sb[:, HEAD + c0 : HEAD + c0 + IG, :],
                ap_abt[c0 : c0 + IG].rearrange("i k f -> k i f"),
            )

        for g in range(BPC // OG):
            o_sb = o_pool.tile([128, OG, MT, N], mybir.dt.float32, tag="o")
            for gi in range(OG):
                i = g * OG + gi
                for mt in range(MT):
                    ps = ps_pool.tile([128, N], mybir.dt.float32, tag="ps")
                    nc.tensor.matmul(
                        ps[:],
                        ab_sb[:, i, mt * 128 : (mt + 1) * 128],
                        ab_sb[:, i, M : M + N],
                        start=True,
                        stop=True,
                    )
                    # Epilogue split across ScalarE and VectorE (each alone
                    # saturates; together they hide under the out-DMA stream).
                    dst = o_sb[:, gi, mt, :]
                    if (i * MT + mt) % 2 == 0:
                        nc.scalar.mul(dst, ps[:], float(alpha))
                    else:
                        nc.vector.tensor_scalar_mul(dst, ps[:], float(alpha))
            # Alternate output DMAs across the two HWDGE queues.
            if g % 2 == 0:
                nc.scalar.dma_start(ap_o[g], o_sb[:])
            else:
                nc.sync.dma_start(ap_o[g], o_sb[:])
    nc.compile()
    return nc


def _get_nc(alpha: float):
    key = np.float32(alpha).tobytes()
    if key not in _cache:
        _cache[key] = _build(alpha)
    return _cache[key]


def kernel(a, b, alpha):
    import ml_dtypes

    from concourse.bass_utils import run_bass_kernel_spmd

    global LAST_RESULTS

    a = np.asarray(a)
    b = np.asarray(b)
    alpha_f = float(np.float32(np.asarray(alpha)))

    # Transpose-pack as int8 so K is the leading (partition) dim on
    # device: [B, M, K] -> [B, K, M]; a and b side by side along the free
    # dim. The device DMA casts int8 -> bf16 (exact for |v| <= 128); the
    # per-core HEAD batches ship pre-cast to bf16 for a fast HWDGE start.
    abT = np.empty((B, K, M + N), dtype=np.int8)
    abT[:, :, :M] = np.swapaxes(a, 1, 2).astype(np.int8, copy=False)
    abT[:, :, M:] = np.swapaxes(b, 1, 2).astype(np.int8, copy=False)

    nc = _get_nc(alpha_f)
    in_maps = [
        {
            "abh": abT[c * BPC : c * BPC + HEAD].astype(ml_dtypes.bfloat16),
            "abt": abT[c * BPC + HEAD : (c + 1) * BPC],
        }
        for c in range(NCORES)
    ]
    res = run_bass_kernel_spmd(nc, in_maps, core_ids=list(range(NCORES)))
    LAST_RESULTS = res
    return np.concatenate([r["out"] for r in res.results], axis=0)


# revision 27
# speedup vs baseline: 1.0858x; 1.0052x over previous
"""Bass/Trainium2 kernel for batched int8 matmul with fp32 dequant epilogue.

Computes out[b, m, n] = alpha * sum_k a[b, m, k] * b[b, n, k] for
a, b int8 [256, 512, 128], out fp32 [256, 512, 512].

Strategy:
  - Shard the batch dim B=256 across 8 NeuronCores (32 batches/core).
  - int8 values convert EXACTLY to bf16 (8-bit significand covers +-256);
    products are ints <= 2^14 and the K=128 accumulation stays <= 2^21,
    exactly representable in the fp32 PSUM accumulator -> the bf16 matmul
    reproduces the int32-accumulated reference bit-exactly.
  - Host pre-transposes both operands to [B, K, M/N] so K lands on the
    SBUF partition dim (the PE contracts over partitions) with fully
    contiguous DMA rows.
  - K=128 means each [128m x 512n] output tile is a single matmul
    (no accumulation loop). alpha is folded into the PSUM->SBUF copy,
    alternating ScalarE/VectorE; fp32 out DMAs back to HBM.
"""

import os
import sys

import numpy as np

B, M, N, K = 256, 512, 512, 128
NCORES = 8
BPC = B // NCORES  # batches per core
MT = M // 128  # m-tiles per batch
OG = 1  # batches per output DMA group (1 batch -> 1 MiB per dma_start)
IG = 4  # batches per input DMA chunk (4 batches -> 1 MiB per dma_start)
HEAD = 8  # leading batches shipped as bf16 and loaded via fast HWDGE

_cache = {}
LAST_RESULTS = None  # BassKernelResults of the most recent run (for profiling)


def _build(alpha: float):
    from contextlib import ExitStack

    import concourse.bass as bass
    import concourse.mybir as mybir
    import concourse.tile as tile
    from concourse import bacc

    nc = bacc.Bacc("TRN2", debug=False, enable_asserts=False, num_devices=NCORES)
    # a and b packed along the free dim so one DMA region feeds both matmul
    # operands. The first HEAD batches ship as bf16 and load via HWDGE
    # (sub-us first byte, so matmuls start ~4us in); the rest ships as
    # int8 (half the HBM read traffic), cast to bf16 inline by the SWDGE
    # input DMAs.
    abh = nc.dram_tensor(
        "abh", [HEAD, K, M + N], mybir.dt.bfloat16, kind="ExternalInput"
    )
    abt = nc.dram_tensor(
        "abt", [BPC - HEAD, K, M + N], mybir.dt.int8, kind="ExternalInput"
    )
    out = nc.dram_tensor("out", [BPC, M, N], mybir.dt.float32, kind="ExternalOutput")

    ap_abh = abh.ap()
    ap_abt = abt.ap()
    # DRAM out viewed with the partition dim innermost of the row index:
    # [BPC, (t p), n] -> [g, p, i, t, n] so one DMA writes OG whole batches
    # from an SBUF tile laid out [p, i, t, n].
    ap_o = out.ap().rearrange("(g i) (t p) n -> g p i t n", p=128, i=OG)

    with ExitStack() as ctx:
        tc = ctx.enter_context(tile.TileContext(nc))
        ab_pool = ctx.enter_context(tc.tile_pool(name="ab", bufs=1))
        ps_pool = ctx.enter_context(tc.tile_pool(name="ps", bufs=8, space="PSUM"))
        wms_pool = ctx.enter_context(tc.tile_pool(name="wms", bufs=1))
        o_pool = ctx.enter_context(tc.tile_pool(name="o", bufs=6))

        # ~7us of dummy back-to-back matmuls at t0 (PE is idle while the
        # first input chunk streams in anyway) to lift the PE HAM clock
        # gate from 1.2 to 2.4 GHz; the steady-state matmul stream then
        # keeps it warm. Cold MMs would otherwise pace the whole pipeline.
        # The warmup PSUM tile cycles through the main pool; its slot is
        # released as soon as the last warmup matmul retires.
        wm_sb = wms_pool.tile([K, 128], mybir.dt.bfloat16, tag="wms")
        nc.vector.memset(wm_sb[:], 0)
        wm_ps = ps_pool.tile([128, N], mybir.dt.float32, tag="ps")
        for _ in range(72):
            nc.tensor.matmul(
                wm_ps[:, 0:128], wm_sb[:], wm_sb[:], start=True, stop=True
            )

        # Whole input resident in SBUF (64KB/partition), streamed in as
        # chunks so the first matmuls start early. The bf16 head goes via
        # HWDGE; the int8 tail via gpsimd (SWDGE) with inline cast, on
        # rings separate from the two HWDGE output queues.
        ab_sb = ab_pool.tile([K, BPC, M + N], mybir.dt.bfloat16, tag="ab")
        half = HEAD // 2
        nc.sync.dma_start(
            ab_sb[:, 0:half, :], ap_abh[0:half].rearrange("i k f -> k i f")
        )
        nc.scalar.dma_start(
            ab_sb[:, half:HEAD, :], ap_abh[half:HEAD].rearrange("i k f -> k i f")
        )
        for c0 in range(0, BPC - HEAD, IG):
            nc.gpsimd.dma_start(
                ab_sb[:, HEAD + c0 : HEAD + c0 + IG, :],
                ap_abt[c0 : c0 + IG].rearrange("i k f -> k i f"),
            )

        for g in range(BPC // OG):
            o_sb = o_pool.tile([128, OG, MT, N], mybir.dt.float32, tag="o")
            for gi in range(OG):
                i = g * OG + gi
                for mt in range(MT):
                    ps = ps_pool.tile([128, N], mybir.dt.float32, tag="ps")
                    nc.tensor.matmul(
                        ps[:],
                        ab_sb[:, i, mt * 128 : (mt + 1) * 128],
                        ab_sb[:, i, M : M + N],
                        start=True,
                        stop=True,
                    )
                    # Epilogue split across ScalarE and VectorE (each alone
                    # saturates; together they hide under the out-DMA stream).
                    dst = o_sb[:, gi, mt, :]
                    if (i * MT + mt) % 2 == 0:
                        nc.scalar.mul(dst, ps[:], float(alpha))
                    else:
                        nc.vector.tensor_scalar_mul(dst, ps[:], float(alpha))
            # Alternate output DMAs across the two HWDGE queues.
            if g % 2 == 0:
                nc.scalar.dma_start(ap_o[g], o_sb[:])
            else:
                nc.sync.dma_start(ap_o[g], o_sb[:])
    nc.compile()
    return nc


def _get_nc(alpha: float):
    key = np.float32(alpha).tobytes()
    if key not in _cache:
        _cache[key] = _build(alpha)
    return _cache[key]


def kernel(a, b, alpha):
    import ml_dtypes

    from concourse.bass_utils import run_bass_kernel_spmd

    global LAST_RESULTS

    a = np.asarray(a)
    b = np.asarray(b)
    alpha_f = float(np.float32(np.asarray(alpha)))

    # Transpose-pack as int8 so K is the leading (partition) dim on
    # device: [B, M, K] -> [B, K, M]; a and b side by side along the free
    # dim. The device DMA casts int8 -> bf16 (exact for |v| <= 128); the
    # per-core HEAD batches ship pre-cast to bf16 for a fast HWDGE start.
    abT = np.empty((B, K, M + N), dtype=np.int8)
    abT[:, :, :M] = np.swapaxes(a, 1, 2).astype(np.int8, copy=False)
    abT[:, :, M:] = np.swapaxes(b, 1, 2).astype(np.int8, copy=False)

    nc = _get_nc(alpha_f)
    in_maps = [
        {
            "abh": abT[c * BPC : c * BPC + HEAD].astype(ml_dtypes.bfloat16),
            "abt": abT[c * BPC + HEAD : (c + 1) * BPC],
        }
        for c in range(NCORES)
    ]
    res = run_bass_kernel_spmd(nc, in_maps, core_ids=list(range(NCORES)))
    LAST_RESULTS = res
    return np.concatenate([r["out"] for r in res.results], axis=0)


# revision 29
# speedup vs baseline: 1.1007x; 1.0137x over previous
"""Bass/Trainium2 kernel for batched int8 matmul with fp32 dequant epilogue.

Computes out[b, m, n] = alpha * sum_k a[b, m, k] * b[b, n, k] for
a, b int8 [256, 512, 128], out fp32 [256, 512, 512].

Strategy:
  - Shard the batch dim B=256 across 8 NeuronCores (32 batches/core).
  - int8 values convert EXACTLY to bf16 (8-bit significand covers +-256);
    products are ints <= 2^14 and the K=128 accumulation stays <= 2^21,
    exactly representable in the fp32 PSUM accumulator -> the bf16 matmul
    reproduces the int32-accumulated reference bit-exactly.
  - Host pre-transposes both operands to [B, K, M/N] so K lands on the
    SBUF partition dim (the PE contracts over partitions) with fully
    contiguous DMA rows.
  - K=128 means each [128m x 512n] output tile is a single matmul
    (no accumulation loop). alpha is folded into the PSUM->SBUF copy,
    alternating ScalarE/VectorE; fp32 out DMAs back to HBM.
"""

import os
import sys

import numpy as np

B, M, N, K = 256, 512, 512, 128
NCORES = 8
BPC = B // NCORES  # batches per core
MT = M // 128  # m-tiles per batch
OG = 1  # batches per output DMA group (1 batch -> 1 MiB per dma_start)
IG = 4  # batches per input DMA chunk (4 batches -> 1 MiB per dma_start)
HEAD = 4  # leading batches shipped as bf16 and loaded via fast HWDGE

_cache = {}
LAST_RESULTS = None  # BassKernelResults of the most recent run (for profiling)


def _build(alpha: float):
    from contextlib import ExitStack

    import concourse.bass as bass
    import concourse.mybir as mybir
    import concourse.tile as tile
    from concourse import bacc

    nc = bacc.Bacc("TRN2", debug=False, enable_asserts=False, num_devices=NCORES)
    # a and b packed along the free dim so one DMA region feeds both matmul
    # operands. The first HEAD batches ship as bf16 and load via HWDGE
    # (sub-us first byte, so matmuls start ~4us in); the rest ships as
    # int8 (half the HBM read traffic), cast to bf16 inline by the SWDGE
    # input DMAs.
    abh = nc.dram_tensor(
        "abh", [HEAD, K, M + N], mybir.dt.bfloat16, kind="ExternalInput"
    )
    abt = nc.dram_tensor(
        "abt", [BPC - HEAD, K, M + N], mybir.dt.int8, kind="ExternalInput"
    )
    out = nc.dram_tensor("out", [BPC, M, N], mybir.dt.float32, kind="ExternalOutput")

    ap_abh = abh.ap()
    ap_abt = abt.ap()
    # DRAM out viewed with the partition dim innermost of the row index:
    # [BPC, (t p), n] -> [g, p, i, t, n] so one DMA writes OG whole batches
    # from an SBUF tile laid out [p, i, t, n].
    ap_o = out.ap().rearrange("(g i) (t p) n -> g p i t n", p=128, i=OG)

    with ExitStack() as ctx:
        tc = ctx.enter_context(tile.TileContext(nc))
        ab_pool = ctx.enter_context(tc.tile_pool(name="ab", bufs=1))
        ps_pool = ctx.enter_context(tc.tile_pool(name="ps", bufs=8, space="PSUM"))
        wms_pool = ctx.enter_context(tc.tile_pool(name="wms", bufs=1))
        o_pool = ctx.enter_context(tc.tile_pool(name="o", bufs=6))

        # ~7us of dummy back-to-back matmuls at t0 (PE is idle while the
        # first input chunk streams in anyway) to lift the PE HAM clock
        # gate from 1.2 to 2.4 GHz; the steady-state matmul stream then
        # keeps it warm. Cold MMs would otherwise pace the whole pipeline.
        # The warmup PSUM tile cycles through the main pool; its slot is
        # released as soon as the last warmup matmul retires.
        wm_sb = wms_pool.tile([K, 128], mybir.dt.bfloat16, tag="wms")
        nc.vector.memset(wm_sb[:], 0)
        wm_ps = ps_pool.tile([128, N], mybir.dt.float32, tag="ps")
        for _ in range(72):
            nc.tensor.matmul(
                wm_ps[:, 0:128], wm_sb[:], wm_sb[:], start=True, stop=True
            )

        # Whole input resident in SBUF (64KB/partition), streamed in as
        # chunks so the first matmuls start early. The bf16 head goes via
        # HWDGE; the int8 tail via gpsimd (SWDGE) with inline cast, on
        # rings separate from the two HWDGE output queues.
        ab_sb = ab_pool.tile([K, BPC, M + N], mybir.dt.bfloat16, tag="ab")
        nc.sync.dma_start(
            ab_sb[:, 0:HEAD, :], ap_abh.rearrange("i k f -> k i f")
        )
        for c0 in range(0, BPC - HEAD, IG):
            nc.gpsimd.dma_start(
                ab_sb[:, HEAD + c0 : HEAD + c0 + IG, :],
                ap_abt[c0 : c0 + IG].rearrange("i k f -> k i f"),
            )

        for g in range(BPC // OG):
            o_sb = o_pool.tile([128, OG, MT, N], mybir.dt.float32, tag="o")
            for gi in range(OG):
                i = g * OG + gi
                for mt in range(MT):
                    ps = ps_pool.tile([128, N], mybir.dt.float32, tag="ps")
                    nc.tensor.matmul(
                        ps[:],
                        ab_sb[:, i, mt * 128 : (mt + 1) * 128],
                        ab_sb[:, i, M : M + N],
                        start=True,
                        stop=True,
                    )
                    # Epilogue split across ScalarE and VectorE (each alone
                    # saturates; together they hide under the out-DMA stream).
                    dst = o_sb[:, gi, mt, :]
                    if (i * MT + mt) % 2 == 0:
                        nc.scalar.mul(dst, ps[:], float(alpha))
                    else:
                        nc.vector.tensor_scalar_mul(dst, ps[:], float(alpha))
            # Alternate output DMAs across the two HWDGE queues.
            if g % 2 == 0:
                nc.scalar.dma_start(ap_o[g], o_sb[:])
            else:
                nc.sync.dma_start(ap_o[g], o_sb[:])
    nc.compile()
    return nc


def _get_nc(alpha: float):
    key = np.float32(alpha).tobytes()
    if key not in _cache:
        _cache[key] = _build(alpha)
    return _cache[key]


def kernel(a, b, alpha):
    import ml_dtypes

    from concourse.bass_utils import run_bass_kernel_spmd

    global LAST_RESULTS

    a = np.asarray(a)
    b = np.asarray(b)
    alpha_f = float(np.float32(np.asarray(alpha)))

    # Transpose-pack as int8 so K is the leading (partition) dim on
    # device: [B, M, K] -> [B, K, M]; a and b side by side along the free
    # dim. The device DMA casts int8 -> bf16 (exact for |v| <= 128); the
    # per-core HEAD batches ship pre-cast to bf16 for a fast HWDGE start.
    abT = np.empty((B, K, M + N), dtype=np.int8)
    abT[:, :, :M] = np.swapaxes(a, 1, 2).astype(np.int8, copy=False)
    abT[:, :, M:] = np.swapaxes(b, 1, 2).astype(np.int8, copy=False)

    nc = _get_nc(alpha_f)
    in_maps = [
        {
            "abh": abT[c * BPC : c * BPC + HEAD].astype(ml_dtypes.bfloat16),
            "abt": abT[c * BPC + HEAD : (c + 1) * BPC],
        }
        for c in range(NCORES)
    ]
    res = run_bass_kernel_spmd(nc, in_maps, core_ids=list(range(NCORES)))
    LAST_RESULTS = res
    return np.concatenate([r["out"] for r in res.results], axis=0)
